# revision 22
# baseline (speedup 1.0000x reference)
"""MultiHeadAttention (cosine-sim, no softmax) + residual + LayerNorm on 8 TRN2 cores.

Reference math (per sample row x of q/k/v, D=2048, H=16, HD=128):
  qp = q @ Wq + bq   (kept as residual)
  kp = k @ Wk + bk ; vp = v @ Wv + bv
  per head h: qn = qh/||qh||, kn = kh/||kh||
  s[h,g] = (qn_h . kn_g) / HD          # [16,16] per sample
  o[h] = sum_g s[h,g] * vh_g           # [16,128]
  o_flat[hd*16+h] = o[h,hd]            # interleaved flatten
  o2 = o_flat @ Wo + bo
  x = qp + o2 ; out = layernorm(x) * gamma + beta

Sharding: pure data-parallel over batch (4096 rows/core), weights replicated.

Device strategy (per core): fully transposed pipeline, bf16 matmuls with
fp32 PSUM accumulation, software-pipelined across 512-sample chunks so the
PE never sits behind the DVE/GpSimd round-trips of the attention/LN phases:

  macro-iteration c emits
    stage 1 (q/k proj+normalize of chunk c)   interleaved with
        attention blocks of chunk c-1 (4 per head group)
    stage 2 (v proj of chunk c)               interleaved with
        o-projection groups, LN stats, normalize+output of chunk c-1

  - Projections per head: psum[128, 512] = sum_ko W[ko,h].T @ xT[ko]
    (weights stationary, host-packed 512KB contiguous DMAs); bias via a
    K=1 matmul.
  - Per-head norm: ACT Square -> ones[128,128] matmul (reduce+broadcast) ->
    ACT Sqrt -> DVE fast reciprocal -> DVE multiply into the interleaved
    [hd, blk, h, s] layout. qnT/knT stored fp8e4 (values in [-1,1]) so both
    chunks' copies fit in SBUF (required for cross-chunk pipelining).
  - Attention per 8 samples: ST[(g,s),(h,s')] = knT8.T @ qnT8, masked by a
    block-diagonal 1/HD constant; V8 = PE-transpose of vhT8;
    oT[hd,(h,s)] = V8.T @ ST_masked.
  - Output projection transposed: o2T[nb] = sum_h Wo'[h,nb].T @ oT[:,h,:];
    residual added in place into qpT (same layout).
  - LayerNorm in transposed space: mean/meansq via two matmul-accumulated
    ones-reductions; gamma/beta as per-partition scalars; normalized bf16
    tiles PE-transposed back to natural layout for contiguous f32 stores.
"""

from contextlib import ExitStack

import numpy as np
import ml_dtypes

import concourse.bass as bass
import concourse.bacc as bacc
import concourse.mybir as mybir
import concourse.tile as tile
from concourse.bass_utils import run_bass_kernel_spmd

BF16 = mybir.dt.bfloat16
F32 = mybir.dt.float32
FP8 = mybir.dt.float8e4
AF = mybir.ActivationFunctionType

B, D, H, HD = 32768, 2048, 16, 128
NCORES = 8
EPS = 1e-5
CHUNK = 512          # samples per chunk
KO = D // 128        # 16 contraction blocks
NB = D // 128        # 16 feature blocks (== heads under d' = h*128+hd)
SBLK = 8             # samples per attention block matmul
NBLK = CHUNK // SBLK
NBT = CHUNK // 128


def build_bass(bl, no_bias=False, unit_gb=False):
    nc = bacc.Bacc()
    nch = bl // CHUNK

    qTd = nc.dram_tensor("qT", [128, nch, KO, CHUNK], BF16, kind="ExternalInput")
    kTd = nc.dram_tensor("kT", [128, nch, KO, CHUNK], BF16, kind="ExternalInput")
    vTd = nc.dram_tensor("vT", [128, nch, KO, CHUNK], BF16, kind="ExternalInput")
    wq = nc.dram_tensor("wq", [128, H, KO, 128], BF16, kind="ExternalInput")
    wk = nc.dram_tensor("wk", [128, H, KO, 128], BF16, kind="ExternalInput")
    wv = nc.dram_tensor("wv", [128, H, KO, 128], BF16, kind="ExternalInput")
    wo = nc.dram_tensor("wo", [128, NB, H, 128], BF16, kind="ExternalInput")
    bqr = nc.dram_tensor("bqr", [1, D], BF16, kind="ExternalInput")
    bkr = nc.dram_tensor("bkr", [1, D], BF16, kind="ExternalInput")
    bvc = nc.dram_tensor("bvc", [128, H], F32, kind="ExternalInput")
    bor = nc.dram_tensor("bor", [1, D], BF16, kind="ExternalInput")
    gpk = nc.dram_tensor("gpk", [128, NB], F32, kind="ExternalInput")
    bpk = nc.dram_tensor("bpk", [128, NB], F32, kind="ExternalInput")
    ident = nc.dram_tensor("ident", [128, 128], BF16, kind="ExternalInput")
    mask = nc.dram_tensor("mask", [128, 128], BF16, kind="ExternalInput")
    ones128 = nc.dram_tensor("ones128", [128, 128], BF16, kind="ExternalInput")
    ones1 = nc.dram_tensor("ones1", [1, CHUNK], BF16, kind="ExternalInput")
    out = nc.dram_tensor("out", [bl, D], F32, kind="ExternalOutput")

    with tile.TileContext(nc) as tc, ExitStack() as ctx:
        consts = ctx.enter_context(tc.tile_pool(name="consts", bufs=1))
        qin = ctx.enter_context(tc.tile_pool(name="qin", bufs=1))
        kin = ctx.enter_context(tc.tile_pool(name="kin", bufs=1))
        vin = ctx.enter_context(tc.tile_pool(name="vin", bufs=1))
        wpool = ctx.enter_context(tc.tile_pool(name="wpool", bufs=4))
        qpT_pool = ctx.enter_context(tc.tile_pool(name="qpT", bufs=2))
        sq_pool = ctx.enter_context(tc.tile_pool(name="sq", bufs=2))
        nrm_pool = ctx.enter_context(tc.tile_pool(name="nrm", bufs=2))
        rs_pool = ctx.enter_context(tc.tile_pool(name="rs", bufs=1))
        nTpool = ctx.enter_context(tc.tile_pool(name="nT", bufs=1))
        oT_pool = ctx.enter_context(tc.tile_pool(name="oT", bufs=1))
        att_sb = ctx.enter_context(tc.tile_pool(name="att_sb", bufs=3))
        ln_pool = ctx.enter_context(tc.tile_pool(name="ln", bufs=2))
        og_pool = ctx.enter_context(tc.tile_pool(name="og", bufs=4))
        pp = ctx.enter_context(tc.tile_pool(
            name="pp", bufs=3 if unit_gb else 2, space="PSUM"))
        ssp = ctx.enter_context(tc.tile_pool(name="ssp", bufs=1, space="PSUM"))
        lnp = ctx.enter_context(tc.tile_pool(name="lnp", bufs=1, space="PSUM"))
        att = ctx.enter_context(tc.tile_pool(
            name="att", bufs=4 if unit_gb else 3, space="PSUM"))

        # ---- constants ----
        ident_sb = consts.tile([128, 128], BF16)
        nc.sync.dma_start(out=ident_sb, in_=ident[:, :])
        mask_sb = consts.tile([128, 128], BF16)
        nc.sync.dma_start(out=mask_sb, in_=mask[:, :])
        ones128_sb = consts.tile([128, 128], BF16)
        nc.sync.dma_start(out=ones128_sb, in_=ones128[:, :])
        ones1_sb = consts.tile([1, CHUNK], BF16)
        nc.sync.dma_start(out=ones1_sb, in_=ones1[:, :])
        bq_sb = consts.tile([1, D], BF16)
        nc.sync.dma_start(out=bq_sb, in_=bqr[:, :])
        bk_sb = consts.tile([1, D], BF16)
        nc.sync.dma_start(out=bk_sb, in_=bkr[:, :])
        bvc_sb = consts.tile([128, H], F32)
        nc.sync.dma_start(out=bvc_sb, in_=bvc[:, :])
        bo_sb = consts.tile([1, D], BF16)
        nc.sync.dma_start(out=bo_sb, in_=bor[:, :])
        g_sb = consts.tile([128, NB], F32)
        nc.sync.dma_start(out=g_sb, in_=gpk[:, :])
        b_sb = consts.tile([128, NB], F32)
        nc.sync.dma_start(out=b_sb, in_=bpk[:, :])
        eps_sb = consts.tile([128, 1], F32)
        nc.vector.memset(eps_sb, EPS)

        st = {}   # per-chunk live tile handles
        filler = []   # deferred PE micro-ops (attention / out-transposes)

        def pop_filler(n):
            for _ in range(n):
                if not filler:
                    return
                filler.pop(0)()

        inputs = {}

        def prefetch_qk(c, engine=None):
            if c >= nch or c in inputs:
                return
            eng = engine or nc.scalar
            qT_sb = qin.tile([128, KO, CHUNK], BF16, tag="qT", name=f"qT{c}")
            eng.dma_start(out=qT_sb, in_=qTd[:, c])
            kT_sb = kin.tile([128, KO, CHUNK], BF16, tag="kT", name=f"kT{c}")
            eng.dma_start(out=kT_sb, in_=kTd[:, c])
            inputs[c] = [qT_sb, kT_sb, None]

        def prefetch_v(c, engine=None):
            if c >= nch or inputs[c][2] is not None:
                return
            eng = engine or nc.scalar
            vT_sb = vin.tile([128, KO, CHUNK], BF16, tag="vT", name=f"vT{c}")
            eng.dma_start(out=vT_sb, in_=vTd[:, c])
            inputs[c][2] = vT_sb
            if c in st:
                st[c]["vT"] = vT_sb

        def start_chunk(c):
            qT_sb, kT_sb, vT_sb = inputs[c]
            st[c] = {
                "qT": qT_sb, "kT": kT_sb, "vT": vT_sb,
                "qnT": nTpool.tile([128, NBLK, H, SBLK], FP8, tag="qnT",
                                   bufs=2, name=f"qnT{c}"),
                "knT": nTpool.tile([128, NBLK, H, SBLK], FP8, tag="knT",
                                   bufs=2, name=f"knT{c}"),
                "vhT": nTpool.tile([128, NBLK, H, SBLK], BF16, tag="vhT",
                                   name=f"vhT{c}"),
                "oT": oT_pool.tile([128, H, CHUNK], BF16, tag="oT",
                                   name=f"oT{c}"),
                "qpT": qpT_pool.tile([128, H, CHUNK], BF16, tag="qpT",
                                     name=f"qpT{c}"),
                "st_t": {}, "vb_sb": {},
            }
            if unit_gb:
                st[c]["xnat"] = og_pool.tile([128, NBT, NB, 128], BF16,
                                             tag="xnat", bufs=1,
                                             name=f"xnat{c}")
                st[c]["stats"] = ln_pool.tile([128, NBT, NB, 6], F32,
                                              tag="stats", bufs=2,
                                              name=f"stats{c}")

        # ---- attention micro-ops (spread through projection streams) ----
        def att_st(c, blk):
            def f():
                s = st[c]
                st_ps = att.tile([128, 128], F32, tag="att",
                                 name=f"st_{c}_{blk}")
                nc.tensor.matmul(
                    st_ps, s["knT"][:, blk].rearrange("p h s -> p (h s)"),
                    s["qnT"][:, blk].rearrange("p h s -> p (h s)"),
                    start=True, stop=True)
                st_t = att_sb.tile([128, 128], BF16, tag="st", bufs=5,
                                   name=f"stb_{c}_{blk}")
                nc.vector.tensor_mul(out=st_t, in0=st_ps, in1=mask_sb)
                s["st_t"][blk] = st_t
            return f

        def att_vb(c, blk):
            def f():
                s = st[c]
                vb_ps = att.tile([128, 128], BF16, tag="att",
                                 name=f"vb_{c}_{blk}")
                nc.tensor.transpose(
                    vb_ps, s["vhT"][:, blk].rearrange("p h s -> p (h s)"),
                    ident_sb)
                vb = att_sb.tile([128, 128], BF16, tag="vb", bufs=5,
                                 name=f"vbs_{c}_{blk}")
                nc.vector.tensor_copy(out=vb, in_=vb_ps)
                s["vb_sb"][blk] = vb
            return f

        def att_o(c, blk):
            def f():
                s = st[c]
                o_ps = att.tile([128, 128], F32, tag="att",
                                name=f"o_{c}_{blk}")
                nc.tensor.matmul(o_ps, s["vb_sb"].pop(blk),
                                 s["st_t"].pop(blk), start=True, stop=True)
                nc.scalar.copy(
                    out=s["oT"][:, :, blk * SBLK:(blk + 1) * SBLK],
                    in_=o_ps.rearrange("p (h s) -> p h s", h=H))
            return f

        def queue_attention(c):
            # block stages pipelined: dependent ops ~6 queue slots apart so
            # the DVE/ACT round-trips between them are fully hidden
            for i in range(NBLK + 4):
                if i < NBLK:
                    filler.append(att_st(c, i))
                if 2 <= i < NBLK + 2:
                    filler.append(att_vb(c, i - 2))
                if 4 <= i:
                    filler.append(att_o(c, i - 4))

        out5 = out.rearrange("(cc bt p) (nb n) -> cc p bt nb n",
                             p=128, bt=NBT, n=128)

        def nat_tp(c, nb, bt):
            # transpose one post-residual x tile to natural layout, stage it
            # and feed per-sample LN stats (natural-LN path)
            def f():
                s = st[c]
                tp = att.tile([128, 128], BF16, tag="att",
                              name=f"tp_{c}_{nb}_{bt}")
                nc.tensor.transpose(tp, s["qpT"][:, nb, bt * 128:(bt + 1) * 128],
                                    ident_sb)
                nc.scalar.copy(out=s["xnat"][:, bt, nb, :], in_=tp)
                nc.vector.bn_stats(out=s["stats"][:, bt, nb, :],
                                   in_=s["xnat"][:, bt, nb, :])
            return f

        def nat_out_bt(c, bt):
            # aggregate stats for one 128-sample block, normalize, store
            s = st[c]
            mv = ln_pool.tile([128, 2], F32, tag="mv", bufs=4,
                              name=f"mv_{c}_{bt}")
            nc.vector.bn_aggr(out=mv, in_=s["stats"][:, bt])
            sd = ln_pool.tile([128, 1], F32, tag="sd1", bufs=4,
                              name=f"sd1_{c}_{bt}")
            nc.scalar.activation(out=sd, in_=mv[:, 1:2], func=AF.Sqrt,
                                 bias=eps_sb)
            rstd = ln_pool.tile([128, 1], F32, tag="rstd1", bufs=4,
                                name=f"rstd1_{c}_{bt}")
            nc.vector.reciprocal(out=rstd, in_=sd)
            b0 = c * CHUNK
            for half in range(2):
                ob = og_pool.tile([128, D // 2], F32, tag="outb", bufs=1,
                                  name=f"ob_{c}_{bt}_{half}")
                for j in range(NB // 2):
                    nb = half * (NB // 2) + j
                    nc.gpsimd.tensor_scalar(
                        out=ob[:, j * 128:(j + 1) * 128],
                        in0=s["xnat"][:, bt, nb, :],
                        scalar1=mv[:, 0:1], scalar2=rstd,
                        op0=mybir.AluOpType.subtract, op1=mybir.AluOpType.mult)
                nc.scalar.dma_start(
                    out=out[b0 + bt * 128:b0 + (bt + 1) * 128,
                            half * (D // 2):(half + 1) * (D // 2)], in_=ob)

        def out_tp(c, nb, bt, d3, og4):
            def f():
                tp = att.tile([128, 128], BF16, tag="att",
                              name=f"tp_{c}_{nb}_{bt}")
                nc.tensor.transpose(tp, d3[:, bt * 128:(bt + 1) * 128],
                                    ident_sb)
                nc.scalar.copy(out=og4[:, bt, :], in_=tp)
                if bt == NBT - 1:
                    nc.scalar.dma_start(out=out5[c, :, :, nb], in_=og4)
            return f

        def emit_qk_head(c, h):
            s = st[c]
            for (xsb, wd, brow, is_q) in ((s["qT"], wq, bq_sb, True),
                                          (s["kT"], wk, bk_sb, False)):
                tag = "q" if is_q else "k"
                wt = wpool.tile([128, KO, 128], BF16, tag="w",
                                name=f"w{tag}_{c}_{h}")
                nc.sync.dma_start(out=wt, in_=wd[:, h])
                ps = pp.tile([128, CHUNK], F32, tag="pp",
                             name=f"p{tag}_{c}_{h}")
                if not no_bias:
                    nc.tensor.matmul(ps, brow[:, h * 128:(h + 1) * 128],
                                     ones1_sb, start=True, stop=False)
                for ko in range(KO):
                    nc.tensor.matmul(ps, wt[:, ko], xsb[:, ko],
                                     start=(no_bias and ko == 0),
                                     stop=(ko == KO - 1))
                    if ko % 4 == 3:
                        pop_filler(2 if ko % 8 == 3 else 1)
                if is_q:
                    nc.scalar.copy(out=s["qpT"][:, h, :], in_=ps)
                sq = sq_pool.tile([128, CHUNK], BF16, tag="sq",
                                  name=f"sq_{c}_{h}_{tag}")
                nc.scalar.activation(out=sq, in_=ps, func=AF.Square)
                ssb = ssp.tile([128, CHUNK], F32, tag="ss",
                               name=f"ss_{c}_{h}_{tag}")
                nc.tensor.matmul(ssb, ones128_sb, sq, start=True, stop=True)
                nrm = nrm_pool.tile([128, CHUNK], F32, tag="nrm",
                                    name=f"nrm_{c}_{h}_{tag}")
                nc.scalar.activation(out=nrm, in_=ssb, func=AF.Sqrt)
                rs = rs_pool.tile([128, CHUNK], F32, tag="rs",
                                  name=f"rs_{c}_{h}_{tag}")
                nc.vector.reciprocal_approx_fast(out=rs, in_=nrm)
                dst = s["qnT"] if is_q else s["knT"]
                nc.vector.tensor_mul(
                    out=dst[:, :, h, :],
                    in0=ps.rearrange("p (blk s) -> p blk s", s=SBLK),
                    in1=rs.rearrange("p (blk s) -> p blk s", s=SBLK))

        def emit_v_head(c, h):
            s = st[c]
            wt = wpool.tile([128, KO, 128], BF16, tag="w", name=f"wv_{c}_{h}")
            nc.sync.dma_start(out=wt, in_=wv[:, h])
            ps = pp.tile([128, CHUNK], F32, tag="pp", name=f"pv_{c}_{h}")
            for ko in range(KO):
                nc.tensor.matmul(ps, wt[:, ko], s["vT"][:, ko],
                                 start=(ko == 0), stop=(ko == KO - 1))
                if ko % 4 == 3:
                    pop_filler(1)
            nc.scalar.activation(
                out=s["vhT"][:, :, h, :],
                in_=ps.rearrange("p (blk s) -> p blk s", s=SBLK),
                func=AF.Identity, bias=bvc_sb[:, h:h + 1])

        def emit_oproj_nb(c, nb, half=None, pops=1):
            s = st[c]
            if nb == 0 and half in (None, 0) and not unit_gb:
                s["sum_ps"] = lnp.tile([128, CHUNK], F32, tag="lnsum",
                                       name=f"lsum_{c}")
                s["sq_ps"] = lnp.tile([128, CHUNK], F32, tag="lnsq",
                                      name=f"lsq_{c}")
            if half is None:
                cs = slice(0, CHUNK)
                first, last = nb == 0, nb == NB - 1
            else:
                cs = slice(half * (CHUNK // 2), (half + 1) * (CHUNK // 2))
                first = nb == 0 and half == 0
                last = nb == NB - 1 and half == 1
            n = cs.stop - cs.start
            wt = wpool.tile([128, H, 128], BF16, tag="w",
                            name=f"wo_{c}_{nb}_{half}")
            nc.sync.dma_start(out=wt, in_=wo[:, nb])
            ps = pp.tile([128, n], F32, tag="pp", name=f"po_{c}_{nb}_{half}")
            if not no_bias:
                nc.tensor.matmul(ps, bo_sb[:, nb * 128:(nb + 1) * 128],
                                 ones1_sb[:, :n], start=True, stop=False)
            for h in range(H):
                nc.tensor.matmul(ps, wt[:, h], s["oT"][:, h, cs],
                                 start=(no_bias and h == 0),
                                 stop=(h == H - 1))
                if h % 4 == 3:
                    pop_filler(pops)
            nc.vector.tensor_add(out=s["qpT"][:, nb, cs], in0=ps,
                                 in1=s["qpT"][:, nb, cs])
            if not unit_gb:
                xsq = sq_pool.tile([128, n], BF16, tag="sq",
                                   name=f"xsq_{c}_{nb}_{half}")
                nc.scalar.activation(out=xsq, in_=s["qpT"][:, nb, cs],
                                     func=AF.Square)
                nc.tensor.matmul(s["sum_ps"][:, cs], ones128_sb,
                                 s["qpT"][:, nb, cs],
                                 start=first, stop=last)
                nc.tensor.matmul(s["sq_ps"][:, cs], ones128_sb, xsq,
                                 start=first, stop=last)

        def emit_ln_stats(c):
            s = st[c]
            mu = ln_pool.tile([128, CHUNK], BF16, tag="mu", name=f"mu_{c}")
            nc.scalar.activation(out=mu, in_=s["sum_ps"], func=AF.Copy,
                                 scale=1.0 / D)
            musq = ln_pool.tile([128, CHUNK], F32, tag="musq", bufs=1,
                                name=f"musq_{c}")
            nc.scalar.activation(out=musq, in_=s["sum_ps"], func=AF.Square,
                                 scale=1.0 / D)
            var = ln_pool.tile([128, CHUNK], F32, tag="var", bufs=1,
                               name=f"var_{c}")
            nc.scalar.activation(out=var, in_=s["sq_ps"], func=AF.Copy,
                                 scale=1.0 / D)
            nc.vector.tensor_sub(out=var, in0=var, in1=musq)
            sd = ln_pool.tile([128, CHUNK], F32, tag="sd", bufs=1,
                              name=f"sd_{c}")
            nc.scalar.activation(out=sd, in_=var, func=AF.Sqrt, bias=eps_sb)
            rstd = ln_pool.tile([128, CHUNK], F32, tag="rstd",
                                name=f"rstd_{c}")
            nc.vector.reciprocal_approx_fast(out=rstd, in_=sd)
            s["mu"], s["rstd"] = mu, rstd

        pending_tp = []

        def emit_out_nb(c, nb, defer=True):
            s = st[c]
            filler.extend(pending_tp)
            del pending_tp[:]
            if unit_gb:
                for bt in range(NBT):
                    f = nat_tp(c, nb, bt)
                    if defer:
                        pending_tp.append(f)
                    else:
                        f()
                return
            d1 = og_pool.tile([128, CHUNK], BF16, tag="d1", bufs=2,
                              name=f"d1_{c}_{nb}")
            nc.vector.tensor_sub(out=d1, in0=s["qpT"][:, nb, :], in1=s["mu"])
            nc.vector.tensor_mul(out=d1, in0=d1, in1=s["rstd"])
            if unit_gb:
                d3 = d1
            else:
                d3 = og_pool.tile([128, CHUNK], BF16, tag="d3", bufs=3,
                                  name=f"d3_{c}_{nb}")
                nc.vector.tensor_scalar(
                    out=d3, in0=d1, scalar1=g_sb[:, nb:nb + 1],
                    scalar2=b_sb[:, nb:nb + 1],
                    op0=mybir.AluOpType.mult, op1=mybir.AluOpType.add)
            og4 = og_pool.tile([128, NBT, 128], F32, tag="og4", bufs=2,
                               name=f"og4_{c}_{nb}")
            for bt in range(NBT):
                f = out_tp(c, nb, bt, d3, og4)
                if defer:
                    pending_tp.append(f)
                else:
                    f()

        def emit_macro(c, prev):
            """stage1+2 of chunk c interleaved (via the filler queue) with
            the attention / oproj / LN / output work of chunk prev."""
            if c is not None:
                start_chunk(c)
                if prev is not None:
                    queue_attention(prev)
                for h in range(H):
                    emit_qk_head(c, h)
                    if c == 0 and h == 1:
                        prefetch_v(0, engine=nc.sync)
                prefetch_qk(c + 1)
                for h in range(H):
                    emit_v_head(c, h)
                    if h == H - 1:
                        prefetch_v(c + 1)
                    if prev is not None:
                        if unit_gb:
                            if h < 8:
                                emit_oproj_nb(prev, 2 * h)
                                emit_oproj_nb(prev, 2 * h + 1)
                                emit_out_nb(prev, 2 * h)
                                emit_out_nb(prev, 2 * h + 1)
                            elif h == 8:
                                filler.extend(pending_tp)
                                del pending_tp[:]
                            elif h == 11:
                                pop_filler(len(filler))
                                nat_out_bt(prev, 0)
                            elif 11 < h < 15:
                                nat_out_bt(prev, h - 11)
                        else:
                            if h < 8:
                                emit_oproj_nb(prev, 2 * h)
                                emit_oproj_nb(prev, 2 * h + 1)
                            elif h == 8:
                                emit_ln_stats(prev)
                                emit_out_nb(prev, 0)
                                emit_out_nb(prev, 1)
                            else:
                                emit_out_nb(prev, 2 * (h - 8))
                                emit_out_nb(prev, 2 * (h - 8) + 1)
                if prev is not None:
                    if unit_gb:
                        pop_filler(len(filler))
                    else:
                        for nb in range(2 * (H - 8), NB):
                            emit_out_nb(prev, nb)
                        filler.extend(pending_tp)
                        del pending_tp[:]
                        pop_filler(len(filler))
            else:
                # flush tail: overlap last chunk's attention with its output
                # projection by splitting oproj groups into sample halves
                queue_attention(prev)
                pop_filler(3 * (NBLK // 2) + 3)   # blocks 0..31 done
                for nb in range(NB):
                    emit_oproj_nb(prev, nb, half=0, pops=2)
                for nb in range(NB):
                    emit_oproj_nb(prev, nb, half=1, pops=2)
                pop_filler(len(filler))
                if unit_gb:
                    for nb in range(NB):
                        emit_out_nb(prev, nb)
                        pop_filler(4)
                    filler.extend(pending_tp)
                    del pending_tp[:]
                    pop_filler(len(filler))
                    for bt in range(NBT):
                        nat_out_bt(prev, bt)
                else:
                    emit_ln_stats(prev)
                    for nb in range(NB):
                        emit_out_nb(prev, nb)
                        pop_filler(4)
                    filler.extend(pending_tp)
                    del pending_tp[:]
                    pop_filler(len(filler))
            if prev is not None:
                del st[prev]
                inputs.pop(prev, None)

        prefetch_qk(0, engine=nc.sync)
        for c in range(nch):
            emit_macro(c, c - 1 if c > 0 else None)
        emit_macro(None, nch - 1)

    nc.compile()
    return nc


def _prep_host_inputs(q, k, v, Wq, bq, Wk, bk, Wv, bv, Wo, bo, gamma, beta,
                      ncores, bl):
    bf = ml_dtypes.bfloat16
    nch = bl // CHUNK

    def pack_xT(x):
        xT = np.ascontiguousarray(x.T).astype(bf)          # [D, B]
        view = xT.reshape(KO, 128, ncores, nch, CHUNK)
        return [np.ascontiguousarray(view[:, :, c].transpose(1, 2, 0, 3))
                for c in range(ncores)]

    def pack_w(W):
        return np.ascontiguousarray(
            W.reshape(KO, 128, H, 128).transpose(1, 2, 0, 3)).astype(bf)

    hh, dd = np.divmod(np.arange(D), HD)
    src = dd * H + hh
    Wo_p = Wo[src, :]
    wo_pack = np.ascontiguousarray(
        Wo_p.reshape(H, 128, NB, 128).transpose(1, 2, 0, 3)).astype(bf)

    r = np.arange(128)
    m = (r[:, None] % SBLK == r[None, :] % SBLK).astype(np.float32) / HD

    shared = {
        "wq": pack_w(Wq), "wk": pack_w(Wk), "wv": pack_w(Wv), "wo": wo_pack,
        "bqr": bq.reshape(1, D).astype(bf),
        "bkr": bk.reshape(1, D).astype(bf),
        "bvc": np.ascontiguousarray(
            bv.reshape(H, 128).T).astype(np.float32),
        "bor": bo.reshape(1, D).astype(bf),
        "gpk": np.ascontiguousarray(
            gamma.reshape(NB, 128).T).astype(np.float32),
        "bpk": np.ascontiguousarray(
            beta.reshape(NB, 128).T).astype(np.float32),
        "ident": np.eye(128, dtype=bf),
        "mask": m.astype(bf),
        "ones128": np.ones((128, 128), dtype=bf),
        "ones1": np.ones((1, CHUNK), dtype=bf),
    }
    return pack_xT(q), pack_xT(k), pack_xT(v), shared


def kernel(q, k, v, Wq, bq, Wk, bk, Wv, bv, Wo, bo, gamma, beta, _bl=None,
           _ncores=None, _trace=False, _tmpdir=None):
    ncores = _ncores or NCORES
    bl = _bl or (q.shape[0] // ncores)
    qTs, kTs, vTs, shared = _prep_host_inputs(
        q, k, v, Wq, bq, Wk, bk, Wv, bv, Wo, bo, gamma, beta, ncores, bl)
    unit_gb = bool(np.all(gamma == 1.0) and not np.any(beta))
    nc = build_bass(bl, no_bias=False, unit_gb=unit_gb)
    in_maps = []
    for c in range(ncores):
        m = dict(shared)
        m["qT"] = qTs[c]
        m["kT"] = kTs[c]
        m["vT"] = vTs[c]
        in_maps.append(m)
    res = run_bass_kernel_spmd(nc, in_maps, core_ids=list(range(ncores)),
                               trace=_trace, tmpdir=_tmpdir)
    outs = [r["out"] for r in res.results]
    full = np.concatenate(outs, axis=0)
    if _trace:
        kernel.last_results = res
    return full.astype(np.float32)


# revision 23
# speedup vs baseline: 1.3166x; 1.3166x over previous
"""MultiHeadAttention (cosine-sim, no softmax) + residual + LayerNorm on 8 TRN2 cores.

Reference math (per sample row x of q/k/v, D=2048, H=16, HD=128):
  qp = q @ Wq + bq   (kept as residual)
  kp = k @ Wk + bk ; vp = v @ Wv + bv
  per head h: qn = qh/||qh||, kn = kh/||kh||
  s[h,g] = (qn_h . kn_g) / HD          # [16,16] per sample
  o[h] = sum_g s[h,g] * vh_g           # [16,128]
  o_flat[hd*16+h] = o[h,hd]            # interleaved flatten
  o2 = o_flat @ Wo + bo
  x = qp + o2 ; out = layernorm(x) * gamma + beta

Sharding: pure data-parallel over batch (4096 rows/core), weights replicated.

Device strategy (per core): fully transposed pipeline, bf16 matmuls with
fp32 PSUM accumulation, software-pipelined across 512-sample chunks so the
PE never sits behind the DVE/GpSimd round-trips of the attention/LN phases:

  macro-iteration c emits
    stage 1 (q/k proj+normalize of chunk c)   interleaved with
        attention blocks of chunk c-1 (4 per head group)
    stage 2 (v proj of chunk c)               interleaved with
        o-projection groups, LN stats, normalize+output of chunk c-1

  - Projections per head: psum[128, 512] = sum_ko W[ko,h].T @ xT[ko]
    (weights stationary, host-packed 512KB contiguous DMAs); bias via a
    K=1 matmul.
  - Per-head norm: ACT Square -> ones[128,128] matmul (reduce+broadcast) ->
    ACT Sqrt -> DVE fast reciprocal -> DVE multiply into the interleaved
    [hd, blk, h, s] layout. qnT/knT stored fp8e4 (values in [-1,1]) so both
    chunks' copies fit in SBUF (required for cross-chunk pipelining).
  - Attention per 8 samples: ST[(g,s),(h,s')] = knT8.T @ qnT8, masked by a
    block-diagonal 1/HD constant; V8 = PE-transpose of vhT8;
    oT[hd,(h,s)] = V8.T @ ST_masked.
  - Output projection transposed: o2T[nb] = sum_h Wo'[h,nb].T @ oT[:,h,:];
    residual added in place into qpT (same layout).
  - LayerNorm in transposed space: mean/meansq via two matmul-accumulated
    ones-reductions; gamma/beta as per-partition scalars; normalized bf16
    tiles PE-transposed back to natural layout for contiguous f32 stores.
"""

from contextlib import ExitStack

import numpy as np
import ml_dtypes

import concourse.bass as bass
import concourse.bacc as bacc
import concourse.mybir as mybir
import concourse.tile as tile
from concourse.bass_utils import run_bass_kernel_spmd

BF16 = mybir.dt.bfloat16
F32 = mybir.dt.float32
FP8 = mybir.dt.float8e4
AF = mybir.ActivationFunctionType

B, D, H, HD = 32768, 2048, 16, 128
NCORES = 8
EPS = 1e-5
CHUNK = 512          # samples per chunk
KO = D // 128        # 16 contraction blocks
NB = D // 128        # 16 feature blocks (== heads under d' = h*128+hd)
SBLK = 8             # samples per attention block matmul
NBLK = CHUNK // SBLK
NBT = CHUNK // 128


def build_bass(bl, no_bias=False, unit_gb=False):
    nc = bacc.Bacc()
    nch = bl // CHUNK

    qTd = nc.dram_tensor("qT", [128, nch, KO, CHUNK], BF16, kind="ExternalInput")
    kTd = nc.dram_tensor("kT", [128, nch, KO, CHUNK], BF16, kind="ExternalInput")
    vTd = nc.dram_tensor("vT", [128, nch, KO, CHUNK], BF16, kind="ExternalInput")
    wq = nc.dram_tensor("wq", [128, H, KO, 128], BF16, kind="ExternalInput")
    wk = nc.dram_tensor("wk", [128, H, KO, 128], BF16, kind="ExternalInput")
    wv = nc.dram_tensor("wv", [128, H, KO, 128], BF16, kind="ExternalInput")
    wo = nc.dram_tensor("wo", [128, NB, H, 128], BF16, kind="ExternalInput")
    bqr = nc.dram_tensor("bqr", [1, D], BF16, kind="ExternalInput")
    bkr = nc.dram_tensor("bkr", [1, D], BF16, kind="ExternalInput")
    bvc = nc.dram_tensor("bvc", [128, H], F32, kind="ExternalInput")
    bor = nc.dram_tensor("bor", [1, D], BF16, kind="ExternalInput")
    gpk = nc.dram_tensor("gpk", [128, NB], F32, kind="ExternalInput")
    bpk = nc.dram_tensor("bpk", [128, NB], F32, kind="ExternalInput")
    ident = nc.dram_tensor("ident", [128, 128], BF16, kind="ExternalInput")
    mask = nc.dram_tensor("mask", [128, 128], BF16, kind="ExternalInput")
    ones128 = nc.dram_tensor("ones128", [128, 128], BF16, kind="ExternalInput")
    ones1 = nc.dram_tensor("ones1", [1, CHUNK], BF16, kind="ExternalInput")
    out = nc.dram_tensor("out", [bl, D], F32, kind="ExternalOutput")

    with tile.TileContext(nc) as tc, ExitStack() as ctx:
        consts = ctx.enter_context(tc.tile_pool(name="consts", bufs=1))
        qin = ctx.enter_context(tc.tile_pool(name="qin", bufs=1))
        kin = ctx.enter_context(tc.tile_pool(name="kin", bufs=1))
        vin = ctx.enter_context(tc.tile_pool(name="vin", bufs=1))
        wpool = ctx.enter_context(tc.tile_pool(name="wpool", bufs=4))
        qpT_pool = ctx.enter_context(tc.tile_pool(name="qpT", bufs=2))
        sq_pool = ctx.enter_context(tc.tile_pool(name="sq", bufs=2))
        nrm_pool = ctx.enter_context(tc.tile_pool(name="nrm", bufs=2))
        rs_pool = ctx.enter_context(tc.tile_pool(name="rs", bufs=2))
        nTpool = ctx.enter_context(tc.tile_pool(name="nT", bufs=1))
        oT_pool = ctx.enter_context(tc.tile_pool(name="oT", bufs=1))
        att_sb = ctx.enter_context(tc.tile_pool(name="att_sb", bufs=3))
        ln_pool = ctx.enter_context(tc.tile_pool(name="ln", bufs=2))
        og_pool = ctx.enter_context(tc.tile_pool(name="og", bufs=4))
        pp = ctx.enter_context(tc.tile_pool(name="pp", bufs=2, space="PSUM"))
        ssp = ctx.enter_context(tc.tile_pool(name="ssp", bufs=1, space="PSUM"))
        lnp = ctx.enter_context(tc.tile_pool(name="lnp", bufs=1, space="PSUM"))
        att = ctx.enter_context(tc.tile_pool(name="att", bufs=3, space="PSUM"))

        # ---- constants ----
        ident_sb = consts.tile([128, 128], BF16)
        nc.sync.dma_start(out=ident_sb, in_=ident[:, :])
        mask_sb = consts.tile([128, 128], BF16)
        nc.sync.dma_start(out=mask_sb, in_=mask[:, :])
        ones128_sb = consts.tile([128, 128], BF16)
        nc.sync.dma_start(out=ones128_sb, in_=ones128[:, :])
        ones1_sb = consts.tile([1, CHUNK], BF16)
        nc.sync.dma_start(out=ones1_sb, in_=ones1[:, :])
        bq_sb = consts.tile([1, D], BF16)
        nc.sync.dma_start(out=bq_sb, in_=bqr[:, :])
        bk_sb = consts.tile([1, D], BF16)
        nc.sync.dma_start(out=bk_sb, in_=bkr[:, :])
        bvc_sb = consts.tile([128, H], F32)
        nc.sync.dma_start(out=bvc_sb, in_=bvc[:, :])
        bo_sb = consts.tile([1, D], BF16)
        nc.sync.dma_start(out=bo_sb, in_=bor[:, :])
        g_sb = consts.tile([128, NB], F32)
        nc.sync.dma_start(out=g_sb, in_=gpk[:, :])
        b_sb = consts.tile([128, NB], F32)
        nc.sync.dma_start(out=b_sb, in_=bpk[:, :])
        eps_sb = consts.tile([128, 1], F32)
        nc.vector.memset(eps_sb, EPS)

        st = {}   # per-chunk live tile handles
        filler = []   # deferred PE micro-ops (attention / out-transposes)

        def pop_filler(n):
            for _ in range(n):
                if not filler:
                    return
                filler.pop(0)()

        inputs = {}

        def prefetch_qk(c, engine=None):
            if c >= nch or c in inputs:
                return
            eng = engine or nc.scalar
            qT_sb = qin.tile([128, KO, CHUNK], BF16, tag="qT", name=f"qT{c}")
            eng.dma_start(out=qT_sb, in_=qTd[:, c])
            kT_sb = kin.tile([128, KO, CHUNK], BF16, tag="kT", name=f"kT{c}")
            eng.dma_start(out=kT_sb, in_=kTd[:, c])
            inputs[c] = [qT_sb, kT_sb, None]

        def prefetch_v(c, engine=None):
            if c >= nch or inputs[c][2] is not None:
                return
            eng = engine or nc.scalar
            vT_sb = vin.tile([128, KO, CHUNK], BF16, tag="vT", name=f"vT{c}")
            eng.dma_start(out=vT_sb, in_=vTd[:, c])
            inputs[c][2] = vT_sb
            if c in st:
                st[c]["vT"] = vT_sb

        def start_chunk(c):
            qT_sb, kT_sb, vT_sb = inputs[c]
            st[c] = {
                "qT": qT_sb, "kT": kT_sb, "vT": vT_sb,
                "qnT": nTpool.tile([128, NBLK, H, SBLK], FP8, tag="qnT",
                                   bufs=2, name=f"qnT{c}"),
                "knT": nTpool.tile([128, NBLK, H, SBLK], FP8, tag="knT",
                                   bufs=2, name=f"knT{c}"),
                "vhT": nTpool.tile([128, NBLK, H, SBLK], BF16, tag="vhT",
                                   name=f"vhT{c}"),
                "oT": oT_pool.tile([128, H, CHUNK], BF16, tag="oT",
                                   name=f"oT{c}"),
                "qpT": qpT_pool.tile([128, H, CHUNK], BF16, tag="qpT",
                                     name=f"qpT{c}"),
                "st_t": {}, "vb_sb": {},
            }

        # ---- attention micro-ops (spread through projection streams) ----
        def att_st(c, blk):
            def f():
                s = st[c]
                st_ps = att.tile([128, 128], F32, tag="att",
                                 name=f"st_{c}_{blk}")
                nc.tensor.matmul(
                    st_ps, s["knT"][:, blk].rearrange("p h s -> p (h s)"),
                    s["qnT"][:, blk].rearrange("p h s -> p (h s)"),
                    start=True, stop=True)
                st_t = att_sb.tile([128, 128], BF16, tag="st", bufs=6,
                                   name=f"stb_{c}_{blk}")
                nc.vector.tensor_mul(out=st_t, in0=st_ps, in1=mask_sb)
                s["st_t"][blk] = st_t
            return f

        def att_vb(c, blk):
            def f():
                s = st[c]
                vb_ps = att.tile([128, 128], BF16, tag="att",
                                 name=f"vb_{c}_{blk}")
                nc.tensor.transpose(
                    vb_ps, s["vhT"][:, blk].rearrange("p h s -> p (h s)"),
                    ident_sb)
                vb = att_sb.tile([128, 128], BF16, tag="vb", bufs=6,
                                 name=f"vbs_{c}_{blk}")
                nc.vector.tensor_copy(out=vb, in_=vb_ps)
                s["vb_sb"][blk] = vb
            return f

        def att_o(c, blk):
            def f():
                s = st[c]
                o_ps = att.tile([128, 128], F32, tag="att",
                                name=f"o_{c}_{blk}")
                nc.tensor.matmul(o_ps, s["vb_sb"].pop(blk),
                                 s["st_t"].pop(blk), start=True, stop=True)
                nc.scalar.copy(
                    out=s["oT"][:, :, blk * SBLK:(blk + 1) * SBLK],
                    in_=o_ps.rearrange("p (h s) -> p h s", h=H))
            return f

        def queue_attention(c):
            # block stages pipelined: dependent ops ~6 queue slots apart so
            # the DVE/ACT round-trips between them are fully hidden
            for i in range(NBLK + 4):
                if i < NBLK:
                    filler.append(att_st(c, i))
                if 2 <= i < NBLK + 2:
                    filler.append(att_vb(c, i - 2))
                if 4 <= i:
                    filler.append(att_o(c, i - 4))

        out5 = out.rearrange("(cc bt p) (nb n) -> cc p bt nb n",
                             p=128, bt=NBT, n=128)

        def out_tp(c, nb, bt, d3, og4):
            def f():
                tp = att.tile([128, 128], BF16, tag="att",
                              name=f"tp_{c}_{nb}_{bt}")
                nc.tensor.transpose(tp, d3[:, bt * 128:(bt + 1) * 128],
                                    ident_sb)
                nc.scalar.copy(out=og4[:, bt, :], in_=tp)
                if bt == NBT - 1:
                    nc.scalar.dma_start(out=out5[c, :, :, nb], in_=og4)
            return f

        def emit_qk_head(c, h):
            s = st[c]
            for (xsb, wd, brow, is_q) in ((s["qT"], wq, bq_sb, True),
                                          (s["kT"], wk, bk_sb, False)):
                tag = "q" if is_q else "k"
                wt = wpool.tile([128, KO, 128], BF16, tag="w",
                                name=f"w{tag}_{c}_{h}")
                nc.sync.dma_start(out=wt, in_=wd[:, h])
                ps = pp.tile([128, CHUNK], F32, tag="pp",
                             name=f"p{tag}_{c}_{h}")
                if not no_bias:
                    nc.tensor.matmul(ps, brow[:, h * 128:(h + 1) * 128],
                                     ones1_sb, start=True, stop=False)
                for ko in range(KO):
                    nc.tensor.matmul(ps, wt[:, ko], xsb[:, ko],
                                     start=(no_bias and ko == 0),
                                     stop=(ko == KO - 1))
                    if ko % 4 == 3:
                        pop_filler(2 if ko % 8 == 3 else 1)
                if is_q:
                    nc.scalar.copy(out=s["qpT"][:, h, :], in_=ps)
                sq = sq_pool.tile([128, CHUNK], BF16, tag="sq",
                                  name=f"sq_{c}_{h}_{tag}")
                nc.scalar.activation(out=sq, in_=ps, func=AF.Square)
                ssb = ssp.tile([128, CHUNK], F32, tag="ss",
                               name=f"ss_{c}_{h}_{tag}")
                nc.tensor.matmul(ssb, ones128_sb, sq, start=True, stop=True)
                nrm = nrm_pool.tile([128, CHUNK], F32, tag="nrm",
                                    name=f"nrm_{c}_{h}_{tag}")
                nc.scalar.activation(out=nrm, in_=ssb, func=AF.Sqrt)
                rs = rs_pool.tile([128, CHUNK], F32, tag="rs",
                                  name=f"rs_{c}_{h}_{tag}")
                nc.vector.reciprocal_approx_fast(out=rs, in_=nrm)
                dst = s["qnT"] if is_q else s["knT"]
                nc.vector.tensor_mul(
                    out=dst[:, :, h, :],
                    in0=ps.rearrange("p (blk s) -> p blk s", s=SBLK),
                    in1=rs.rearrange("p (blk s) -> p blk s", s=SBLK))

        def emit_v_head(c, h):
            s = st[c]
            wt = wpool.tile([128, KO, 128], BF16, tag="w", name=f"wv_{c}_{h}")
            nc.sync.dma_start(out=wt, in_=wv[:, h])
            ps = pp.tile([128, CHUNK], F32, tag="pp", name=f"pv_{c}_{h}")
            for ko in range(KO):
                nc.tensor.matmul(ps, wt[:, ko], s["vT"][:, ko],
                                 start=(ko == 0), stop=(ko == KO - 1))
                if ko % 4 == 3:
                    pop_filler(1)
            nc.scalar.activation(
                out=s["vhT"][:, :, h, :],
                in_=ps.rearrange("p (blk s) -> p blk s", s=SBLK),
                func=AF.Identity, bias=bvc_sb[:, h:h + 1])

        def emit_oproj_nb(c, nb, half=None, pops=1):
            s = st[c]
            if nb == 0 and half in (None, 0):
                s["sum_ps"] = lnp.tile([128, CHUNK], F32, tag="lnsum",
                                       name=f"lsum_{c}")
                s["sq_ps"] = lnp.tile([128, CHUNK], F32, tag="lnsq",
                                      name=f"lsq_{c}")
            if half is None:
                cs = slice(0, CHUNK)
                first, last = nb == 0, nb == NB - 1
            else:
                cs = slice(half * (CHUNK // 2), (half + 1) * (CHUNK // 2))
                first = nb == 0 and half == 0
                last = nb == NB - 1 and half == 1
            n = cs.stop - cs.start
            wt = wpool.tile([128, H, 128], BF16, tag="w",
                            name=f"wo_{c}_{nb}_{half}")
            nc.sync.dma_start(out=wt, in_=wo[:, nb])
            ps = pp.tile([128, n], F32, tag="pp", name=f"po_{c}_{nb}_{half}")
            if not no_bias:
                nc.tensor.matmul(ps, bo_sb[:, nb * 128:(nb + 1) * 128],
                                 ones1_sb[:, :n], start=True, stop=False)
            for h in range(H):
                nc.tensor.matmul(ps, wt[:, h], s["oT"][:, h, cs],
                                 start=(no_bias and h == 0),
                                 stop=(h == H - 1))
                if h % 4 == 3:
                    pop_filler(pops)
            nc.vector.tensor_add(out=s["qpT"][:, nb, cs], in0=ps,
                                 in1=s["qpT"][:, nb, cs])
            xsq = sq_pool.tile([128, n], BF16, tag="sq",
                               name=f"xsq_{c}_{nb}_{half}")
            nc.scalar.activation(out=xsq, in_=s["qpT"][:, nb, cs],
                                 func=AF.Square)
            nc.tensor.matmul(s["sum_ps"][:, cs], ones128_sb,
                             s["qpT"][:, nb, cs],
                             start=first, stop=last)
            nc.tensor.matmul(s["sq_ps"][:, cs], ones128_sb, xsq,
                             start=first, stop=last)

        def emit_ln_stats(c):
            s = st[c]
            mu = ln_pool.tile([128, CHUNK], BF16, tag="mu", name=f"mu_{c}")
            nc.scalar.activation(out=mu, in_=s["sum_ps"], func=AF.Copy,
                                 scale=1.0 / D)
            musq = ln_pool.tile([128, CHUNK], F32, tag="musq", bufs=1,
                                name=f"musq_{c}")
            nc.scalar.activation(out=musq, in_=s["sum_ps"], func=AF.Square,
                                 scale=1.0 / D)
            var = ln_pool.tile([128, CHUNK], F32, tag="var", bufs=1,
                               name=f"var_{c}")
            nc.scalar.activation(out=var, in_=s["sq_ps"], func=AF.Copy,
                                 scale=1.0 / D)
            nc.vector.tensor_sub(out=var, in0=var, in1=musq)
            sd = ln_pool.tile([128, CHUNK], F32, tag="sd", bufs=1,
                              name=f"sd_{c}")
            nc.scalar.activation(out=sd, in_=var, func=AF.Sqrt, bias=eps_sb)
            rstd = ln_pool.tile([128, CHUNK], F32, tag="rstd",
                                name=f"rstd_{c}")
            nc.vector.reciprocal_approx_fast(out=rstd, in_=sd)
            s["mu"], s["rstd"] = mu, rstd

        pending_tp = []

        def emit_out_nb(c, nb, defer=True):
            s = st[c]
            filler.extend(pending_tp)
            del pending_tp[:]
            d1 = og_pool.tile([128, CHUNK], BF16, tag="d1", bufs=2,
                              name=f"d1_{c}_{nb}")
            nc.vector.tensor_sub(out=d1, in0=s["qpT"][:, nb, :], in1=s["mu"])
            nc.vector.tensor_mul(out=d1, in0=d1, in1=s["rstd"])
            if unit_gb:
                d3 = d1
            else:
                d3 = og_pool.tile([128, CHUNK], BF16, tag="d3", bufs=3,
                                  name=f"d3_{c}_{nb}")
                nc.vector.tensor_scalar(
                    out=d3, in0=d1, scalar1=g_sb[:, nb:nb + 1],
                    scalar2=b_sb[:, nb:nb + 1],
                    op0=mybir.AluOpType.mult, op1=mybir.AluOpType.add)
            og4 = og_pool.tile([128, NBT, 128], F32, tag="og4", bufs=2,
                               name=f"og4_{c}_{nb}")
            for bt in range(NBT):
                f = out_tp(c, nb, bt, d3, og4)
                if defer:
                    pending_tp.append(f)
                else:
                    f()

        def emit_macro(c, prev):
            """stage1+2 of chunk c interleaved (via the filler queue) with
            the attention / oproj / LN / output work of chunk prev."""
            if c is not None:
                start_chunk(c)
                if prev is not None:
                    queue_attention(prev)
                for h in range(H):
                    emit_qk_head(c, h)
                    if c == 0 and h == 1:
                        prefetch_v(0, engine=nc.sync)
                prefetch_qk(c + 1)
                for h in range(H):
                    emit_v_head(c, h)
                    if h == H - 1:
                        prefetch_v(c + 1)
                    if prev is not None:
                        if h < 8:
                            emit_oproj_nb(prev, 2 * h)
                            emit_oproj_nb(prev, 2 * h + 1)
                        elif h == 8:
                            emit_ln_stats(prev)
                            emit_out_nb(prev, 0)
                            emit_out_nb(prev, 1)
                        else:
                            emit_out_nb(prev, 2 * (h - 8))
                            emit_out_nb(prev, 2 * (h - 8) + 1)
                if prev is not None:
                    for nb in range(2 * (H - 8), NB):
                        emit_out_nb(prev, nb)
                    filler.extend(pending_tp)
                    del pending_tp[:]
                    pop_filler(len(filler))
            else:
                # flush tail: overlap last chunk's attention with its output
                # projection by splitting oproj groups into sample halves
                queue_attention(prev)
                pop_filler(3 * (NBLK // 2) + 3)   # blocks 0..31 done
                for nb in range(NB):
                    emit_oproj_nb(prev, nb, half=0, pops=2)
                for nb in range(NB):
                    emit_oproj_nb(prev, nb, half=1, pops=2)
                pop_filler(len(filler))
                emit_ln_stats(prev)
                for nb in range(NB):
                    emit_out_nb(prev, nb)
                    pop_filler(4)
                filler.extend(pending_tp)
                del pending_tp[:]
                pop_filler(len(filler))
            if prev is not None:
                del st[prev]
                inputs.pop(prev, None)

        prefetch_qk(0, engine=nc.sync)
        for c in range(nch):
            emit_macro(c, c - 1 if c > 0 else None)
        emit_macro(None, nch - 1)

    nc.compile()
    return nc


def _prep_host_inputs(q, k, v, Wq, bq, Wk, bk, Wv, bv, Wo, bo, gamma, beta,
                      ncores, bl):
    bf = ml_dtypes.bfloat16
    nch = bl // CHUNK

    def pack_xT(x):
        xT = np.ascontiguousarray(x.T).astype(bf)          # [D, B]
        view = xT.reshape(KO, 128, ncores, nch, CHUNK)
        return [np.ascontiguousarray(view[:, :, c].transpose(1, 2, 0, 3))
                for c in range(ncores)]

    def pack_w(W):
        return np.ascontiguousarray(
            W.reshape(KO, 128, H, 128).transpose(1, 2, 0, 3)).astype(bf)

    hh, dd = np.divmod(np.arange(D), HD)
    src = dd * H + hh
    Wo_p = Wo[src, :]
    wo_pack = np.ascontiguousarray(
        Wo_p.reshape(H, 128, NB, 128).transpose(1, 2, 0, 3)).astype(bf)

    r = np.arange(128)
    m = (r[:, None] % SBLK == r[None, :] % SBLK).astype(np.float32) / HD

    shared = {
        "wq": pack_w(Wq), "wk": pack_w(Wk), "wv": pack_w(Wv), "wo": wo_pack,
        "bqr": bq.reshape(1, D).astype(bf),
        "bkr": bk.reshape(1, D).astype(bf),
        "bvc": np.ascontiguousarray(
            bv.reshape(H, 128).T).astype(np.float32),
        "bor": bo.reshape(1, D).astype(bf),
        "gpk": np.ascontiguousarray(
            gamma.reshape(NB, 128).T).astype(np.float32),
        "bpk": np.ascontiguousarray(
            beta.reshape(NB, 128).T).astype(np.float32),
        "ident": np.eye(128, dtype=bf),
        "mask": m.astype(bf),
        "ones128": np.ones((128, 128), dtype=bf),
        "ones1": np.ones((1, CHUNK), dtype=bf),
    }
    return pack_xT(q), pack_xT(k), pack_xT(v), shared


def kernel(q, k, v, Wq, bq, Wk, bk, Wv, bv, Wo, bo, gamma, beta, _bl=None,
           _ncores=None, _trace=False, _tmpdir=None):
    ncores = _ncores or NCORES
    bl = _bl or (q.shape[0] // ncores)
    qTs, kTs, vTs, shared = _prep_host_inputs(
        q, k, v, Wq, bq, Wk, bk, Wv, bv, Wo, bo, gamma, beta, ncores, bl)
    unit_gb = bool(np.all(gamma == 1.0) and not np.any(beta))
    nc = build_bass(bl, no_bias=False, unit_gb=unit_gb)
    in_maps = []
    for c in range(ncores):
        m = dict(shared)
        m["qT"] = qTs[c]
        m["kT"] = kTs[c]
        m["vT"] = vTs[c]
        in_maps.append(m)
    res = run_bass_kernel_spmd(nc, in_maps, core_ids=list(range(ncores)),
                               trace=_trace, tmpdir=_tmpdir)
    outs = [r["out"] for r in res.results]
    full = np.concatenate(outs, axis=0)
    if _trace:
        kernel.last_results = res
    return full.astype(np.float32)
